# revision 7
# baseline (speedup 1.0000x reference)
"""Trainium2 Bass kernel for a Tacotron2-style decoder cell.

Strategy: data-parallel over batch B=64 across 8 NeuronCores (8 rows each),
all weights replicated in bf16.  Everything streamed through the PE in bf16,
accumulation + epilogues in f32.

Math reformulations (bit-compatible within bf16 tolerance):
  * location conv + location fc are fused:  Wcomb = W_lfc @ W_loc.reshape(F, 2K)
    so  loc_fc[b,t,:] = Wcomb @ im2col(aw)[b,:,t]  -- one matmul instead of two.
  * softmax computed as exp(e)/sum(exp(e)) without max subtraction (|e| <~ 3).
  * masked energies = energies + (mask ? -1e30 : 0), injected into the energies
    PSUM accumulation with an identity matmul.
  * biases folded into an extra k=1 matmul with a ones lhsT row.
"""

import numpy as np
import ml_dtypes

import concourse.bass as bass
import concourse.mybir as mybir
import concourse.tile as tile
from concourse import bacc
from concourse.bass_utils import run_bass_kernel_spmd

BF16 = mybir.dt.bfloat16
F32 = mybir.dt.float32
NEG_INF = -1e30

# ----------------------------------------------------------------------------
# configuration
# ----------------------------------------------------------------------------

FULL_CFG = dict(
    B=64, T=2048, E=256, A=128, P=128, R=1024, D=1024, H=256, F=32, K=31,
    n_cores=8,
)


def _derived(cfg):
    c = dict(cfg)
    c["b"] = c["B"] // c["n_cores"]       # local batch rows per core
    c["PAD"] = (c["K"] - 1) // 2
    c["CK"] = 2 * c["K"]                  # im2col rows
    c["G1"] = 4 * c["R"]                  # attention lstm gate width
    c["G2"] = 4 * c["D"]                  # decoder lstm gate width
    c["X1"] = c["P"] + c["E"]             # attention lstm input width
    c["X2"] = c["R"] + c["E"]             # decoder lstm input width (ah|ctx)
    c["HO"] = c["H"] + 1                  # proj columns + gate column
    for k in ("R", "D", "E", "X1"):
        assert c[k] % 128 == 0
    assert c["A"] == 128 and c["T"] % 512 == 0 and c["b"] <= 8
    return c


# ----------------------------------------------------------------------------
# host-side packing helpers
# ----------------------------------------------------------------------------

def _bf(x):
    return np.ascontiguousarray(np.asarray(x, np.float32)).astype(ml_dtypes.bfloat16)


def _f32(x):
    return np.ascontiguousarray(np.asarray(x, np.float32))


def _pack(x):
    """[b, n*128] row-major state -> [128, n*b] 'pack' layout.

    pack[p, c*b + j] = x[j, c*128 + p]  (chunk-major columns, batch minor)."""
    b, w = x.shape
    n = w // 128
    return np.ascontiguousarray(
        x.reshape(b, n, 128).transpose(2, 1, 0).reshape(128, n * b))


def make_in_maps(inputs, cfg):
    """Shard + pre-pack all inputs.  Returns list of per-core input dicts."""
    c = _derived(cfg)
    b, T, E, A, R, D, F, K = (c[k] for k in ("b", "T", "E", "A", "R", "D", "F", "K"))
    ncore = c["n_cores"]

    i = {k: np.asarray(v) for k, v in inputs.items()}

    # ---- replicated weights ----
    W_a = _bf(np.concatenate([i["W_ih_a"].T, i["W_hh_a"].T], axis=0))      # [X1+R, G1]
    b_a = _bf((i["b_ih_a"] + i["b_hh_a"])[None, :])                        # [1, G1]
    W_d = _bf(np.concatenate([i["W_ih_d"].T, i["W_hh_d"].T], axis=0))      # [R+E+D, G2]
    b_d = _bf((i["b_ih_d"] + i["b_hh_d"])[None, :])                        # [1, G2]
    W_o = _bf(np.concatenate([i["W_proj"], i["W_gate"]], axis=0).T)        # [D+E, H+1]
    b_o = _bf(np.concatenate([i["b_proj"], i["b_gate"]])[None, :])         # [1, H+1]
    W_qT = _bf(np.asarray(i["W_q"]).T)                                     # [R, A]
    Wcomb = np.asarray(i["W_lfc"], np.float32) @ np.asarray(
        i["W_loc"], np.float32).reshape(F, 2 * K)                          # [A, 2K]
    WcombT = _bf(Wcomb.T)                                                  # [2K, A]
    Wv_bcols = np.zeros((A, 8 * b), np.float32)
    for j in range(b):
        Wv_bcols[:, j * b + j] = np.asarray(i["W_v"], np.float32)[0]
    Wv_bcols = _bf(Wv_bcols[:, : b * b])                                   # [A, b*b]
    eye_bf = _bf(np.eye(128, dtype=np.float32))                            # [128,128]
    eye_f = _f32(np.eye(128, dtype=np.float32))
    maskrows = np.zeros((b, b * E), np.uint8)
    for j in range(b):
        maskrows[j, j * E:(j + 1) * E] = 1                                 # [b, b*E]

    rep = dict(W_a=W_a, b_a=b_a, W_d=W_d, b_d=b_d, W_o=W_o, b_o=b_o,
               W_qT=W_qT, WcombT=WcombT, Wv_bcols=Wv_bcols,
               eye_bf=eye_bf, eye_f=eye_f, maskrows=maskrows)

    in_maps = []
    for core in range(ncore):
        rs = slice(core * b, (core + 1) * b)
        x1 = np.concatenate([i["last_frame"][rs], i["att_ctx"][rs]], axis=1)
        aw_pad = np.zeros((b, 2, T + 2 * c["PAD"]), np.float32)
        aw_pad[:, 0, c["PAD"]:c["PAD"] + T] = i["att_w"][rs]
        aw_pad[:, 1, c["PAD"]:c["PAD"] + T] = i["att_w_cum"][rs]
        maskadd = np.where(i["mask"][rs], NEG_INF, 0.0).astype(np.float32)
        m = dict(
            x1_pack=_bf(_pack(x1)),
            h1_pack=_bf(_pack(_f32(i["att_h"][rs]))),
            hd_pack=_bf(_pack(_f32(i["dec_h"][rs]))),
            c1_bm=_f32(i["att_c"][rs]),
            cd_bm=_f32(i["dec_c"][rs]),
            aw_pad=_bf(aw_pad),
            pm_T=_bf(np.asarray(i["processed_memory"][rs]).transpose(0, 2, 1)),
            mem=_bf(i["memory"][rs]),
            maskadd=_bf(maskadd),
        )
        m.update(rep)
        in_maps.append(m)
    return in_maps


def gather_outputs(results, cfg):
    c = _derived(cfg)
    H = c["H"]
    out1 = np.concatenate([r["out1"] for r in results], axis=0)   # [B, H+1]
    out2 = np.concatenate([r["out2"] for r in results], axis=0)   # [B, T]
    decoder_output = np.ascontiguousarray(out1[:, :H])
    stop = np.ascontiguousarray(out1[:, H:H + 1])
    weights = np.ascontiguousarray(out2)[:, None, :]
    return decoder_output, stop, weights


# ----------------------------------------------------------------------------
# kernel builder
# ----------------------------------------------------------------------------

def build_kernel(cfg):
    c = _derived(cfg)
    b, T, E, A, R, D = (c[k] for k in ("b", "T", "E", "A", "R", "D"))
    X1, X2, G1, G2, HO, CK, PAD = (c[k] for k in ("X1", "X2", "G1", "G2", "HO", "CK", "PAD"))
    TP = T + 2 * PAD
    NT = T // 512            # 512-wide t chunks
    NC = T // 128            # 128-wide t chunks
    NE = E // 128            # 128-wide e chunks
    K1 = X1 // 128           # x1 k-chunks
    KH1 = R // 128           # h k-chunks (lstm1)
    KH2 = D // 128
    KC = E // 128            # ctx k-chunks
    Sig = mybir.ActivationFunctionType.Sigmoid
    Tanh = mybir.ActivationFunctionType.Tanh
    Exp = mybir.ActivationFunctionType.Exp
    Copy = mybir.ActivationFunctionType.Copy

    nc = bacc.Bacc("TRN2", target_bir_lowering=False, debug=False)

    def din(name, shape, dt=BF16):
        return nc.declare_dram_parameter(name, list(shape), dt, isOutput=False)

    # inputs
    x1_pack = din("x1_pack", [128, K1 * b])
    h1_pack = din("h1_pack", [128, KH1 * b])
    hd_pack = din("hd_pack", [128, KH2 * b])
    c1_bm = din("c1_bm", [b, R], F32)
    cd_bm = din("cd_bm", [b, D], F32)
    aw_pad = din("aw_pad", [b, 2, TP])
    pm_T = din("pm_T", [b, A, T])
    mem = din("mem", [b, T, E])
    maskadd = din("maskadd", [b, T])
    W_a = din("W_a", [X1 + R, G1])
    b_a = din("b_a", [1, G1])
    W_d = din("W_d", [X2 + D, G2])
    b_d = din("b_d", [1, G2])
    W_o = din("W_o", [D + E, HO])
    b_o = din("b_o", [1, HO])
    W_qT = din("W_qT", [R, A])
    WcombT = din("WcombT", [CK, A])
    Wv_bcols = din("Wv_bcols", [A, b * b])
    eye_bf = din("eye_bf", [128, 128])
    eye_f = din("eye_f", [128, 128], F32)
    maskrows = din("maskrows", [b, b * E], mybir.dt.uint8)

    out1 = nc.declare_dram_parameter("out1", [b, HO], F32, isOutput=True)
    out2 = nc.declare_dram_parameter("out2", [b, T], F32, isOutput=True)

    with tile.TileContext(nc) as tc:
        _body(nc, tc, c, locals())
    nc.compile()
    return nc


def _body(nc, tc, c, t):
    b, T, E, A, R, D = (c[k] for k in ("b", "T", "E", "A", "R", "D"))
    X1, X2, G1, G2, HO, CK, PAD = (c[k] for k in ("X1", "X2", "G1", "G2", "HO", "CK", "PAD"))
    TP = T + 2 * PAD
    NT, NC, NE = T // 512, T // 128, E // 128
    K1, KH1, KH2, KC = X1 // 128, R // 128, D // 128, E // 128
    Sig = mybir.ActivationFunctionType.Sigmoid
    Tanh = mybir.ActivationFunctionType.Tanh
    Exp = mybir.ActivationFunctionType.Exp
    Copy = mybir.ActivationFunctionType.Copy

    HB = min(c["G1"], c["G2"], 2048)      # lstm psum half-block columns
    assert c["G1"] % HB == 0 and c["G2"] % HB == 0

    import contextlib
    ctx = contextlib.ExitStack()
    with ctx:
        # ---------------- persistent SBUF pools ----------------
        consts = ctx.enter_context(tc.tile_pool(name="consts", bufs=1))
        wpool = ctx.enter_context(tc.tile_pool(name="wpool", bufs=3))
        gpool = ctx.enter_context(tc.tile_pool(name="gpool", bufs=1))
        spool = ctx.enter_context(tc.tile_pool(name="spool", bufs=1))
        mempool = ctx.enter_context(tc.tile_pool(name="mempool", bufs=2))
        pmpool = ctx.enter_context(tc.tile_pool(name="pmpool", bufs=2))
        impool = ctx.enter_context(tc.tile_pool(name="impool", bufs=2))
        tanhpool = ctx.enter_context(tc.tile_pool(name="tanhpool", bufs=3))

        dma = nc.sync.dma_start

        # ---------------- constants / small inputs ----------------
        def lc(name, shape, dt=BF16, src=None):
            tl = consts.tile(list(shape), dt, name=name, tag=name)
            dma(tl[:], (t[name] if src is None else src)[:])
            return tl

        x1p = lc("x1_pack", [128, K1 * b])
        h1p = lc("h1_pack", [128, KH1 * b])
        hdp = lc("hd_pack", [128, KH2 * b])
        c1s = lc("c1_bm", [b, R], F32)
        cds = lc("cd_bm", [b, D], F32)
        madd = lc("maskadd", [b, T])
        wq = consts.tile([128, KH1 * A], BF16, name="wq", tag="wq")
        dma(wq[:], bass.AP(t["W_qT"], 0, [[A, 128], [128 * A, KH1], [1, A]]))
        wcomb = lc("WcombT", [CK, A])
        wvb = lc("Wv_bcols", [A, b * b])
        ibf = lc("eye_bf", [128, 128])
        ief = lc("eye_f", [128, 128], F32)
        mrows = lc("maskrows", [b, b * E], mybir.dt.uint8)
        wo = consts.tile([128, ((D + E) // 128) * HO], BF16, name="wo", tag="wo")
        dma(wo[:], bass.AP(t["W_o"], 0, [[HO, 128], [128 * HO, (D + E) // 128], [1, HO]]))
        ba = lc("b_a", [1, G1])
        bd = lc("b_d", [1, G2])
        bo = lc("b_o", [1, HO])
        ones = consts.tile([1, 8], BF16, name="ones", tag="ones")
        nc.vector.memset(ones[:], 1.0)

        # ---------------- generic lstm ----------------
        def lstm(xchunks, W_dram, b_sb, G, c_sb, KTOT, pspool):
            """xchunks: list of KTOT [128,b] lhsT APs. Returns h_bm [b, G/4] f32."""
            q = G // 4
            gact = gpool.tile([b, G], F32, name="gact", tag="gact")
            for hf in range(G // HB):
                ps = pspool.tile([b, HB], F32, name="ps_g", tag="ps_g")
                for kc in range(KTOT):
                    wt = wpool.tile([128, HB], BF16, name="wt", tag="w")
                    dma(wt[:], t[W_dram][kc * 128:(kc + 1) * 128,
                                         hf * HB:(hf + 1) * HB])
                    for nb in range(HB // 512):
                        nc.tensor.matmul(ps[:, nb * 512:(nb + 1) * 512], xchunks[kc],
                                         wt[:, nb * 512:(nb + 1) * 512],
                                         start=(kc == 0), stop=False)
                for nb in range(HB // 512):
                    g0 = hf * HB + nb * 512
                    nc.tensor.matmul(ps[:, nb * 512:(nb + 1) * 512], ones[0:1, 0:b],
                                     b_sb[0:1, g0:g0 + 512], start=False, stop=True)
                    func = Tanh if g0 // q == 2 else Sig
                    nc.scalar.activation(gact[:, g0:g0 + 512],
                                         ps[:, nb * 512:(nb + 1) * 512], func)
            sig_i, sig_f = gact[:, 0:q], gact[:, q:2 * q]
            tanh_g, sig_o = gact[:, 2 * q:3 * q], gact[:, 3 * q:4 * q]
            t1 = spool.tile([b, q], F32, name="t1", tag="t1")
            t2 = spool.tile([b, q], F32, name="t2", tag="t2")
            c2 = spool.tile([b, q], F32, name="c2", tag="c2")
            h_bm = spool.tile([b, q], F32, name=f"h_{W_dram}", tag=f"h_{W_dram}")
            nc.vector.tensor_mul(t1[:], sig_i, tanh_g)
            nc.vector.tensor_mul(t2[:], sig_f, c_sb[:])
            nc.vector.tensor_add(c2[:], t1[:], t2[:])
            nc.scalar.activation(t1[:], c2[:], Tanh)
            nc.vector.tensor_mul(h_bm[:], sig_o, t1[:])
            return h_bm

        def transpose_pack(src_bm, n128, dst_bf, pool):
            """src_bm [b, n128*128] f32 -> dst pack tile [128, n128*b] bf16."""
            for cc in range(n128):
                pt = pool.tile([128, b], F32, name="pt", tag="pt")
                nc.tensor.transpose(pt[:], src_bm[:, cc * 128:(cc + 1) * 128],
                                    ief[0:b, 0:b])
                nc.scalar.activation(dst_bf[:, cc * b:(cc + 1) * b], pt[:], Copy)

        # ================= phase A: attention LSTM =================
        ah_pack = consts.tile([128, KH1 * b], BF16, name="ah_pack", tag="ah_pack")
        with tc.tile_pool(name="psA1", bufs=1, space="PSUM") as psA1, \
             tc.tile_pool(name="psT1", bufs=2, space="PSUM") as psT1:
            x1chunks = [x1p[:, kc * b:(kc + 1) * b] for kc in range(K1)] + \
                       [h1p[:, kc * b:(kc + 1) * b] for kc in range(KH1)]
            ah_bm = lstm(x1chunks, "W_a", ba, G1, c1s, K1 + KH1, psA1)
            transpose_pack(ah_bm, KH1, ah_pack, psT1)

        # ================= phase B: attention =================
        wT = consts.tile([128, NC * b], BF16, name="wT", tag="wT")
        w_sb = spool.tile([b, T], F32, name="w_sb", tag="w_sb")
        with tc.tile_pool(name="psB", bufs=2, space="PSUM") as psB, \
             tc.tile_pool(name="psC", bufs=1, space="PSUM") as psC, \
             tc.tile_pool(name="psT2", bufs=2, space="PSUM") as psT2:
            # pq = W_q @ ah  -> [A, b]
            pq_ps = psT2.tile([128, b], F32, name="pt", tag="pt")
            for kc in range(KH1):
                nc.tensor.matmul(pq_ps[:], wq[:, kc * A:(kc + 1) * A],
                                 ah_pack[:, kc * b:(kc + 1) * b],
                                 start=(kc == 0), stop=(kc == KH1 - 1))
            pq_sb = spool.tile([128, b], F32, name="pq", tag="pq")
            nc.scalar.activation(pq_sb[:], pq_ps[:], Copy)

            # energies psum [b, T], seeded with the additive mask
            psE_t = psC.tile([b, T], F32, name="psE_t")
            for tcn in range(NT):
                nc.tensor.matmul(psE_t[:, tcn * 512:(tcn + 1) * 512], ibf[0:b, 0:b],
                                 madd[:, tcn * 512:(tcn + 1) * 512],
                                 start=True, stop=False)
            for bb in range(b):
                imc = impool.tile([CK, T], BF16, name="imc", tag="imc")
                dma(imc[:], bass.AP(t["aw_pad"], bb * 2 * TP,
                                    [[TP, 2], [1, c["K"]], [1, T]]))
                pmb = pmpool.tile([A, T], BF16, name="pmb", tag="pm")
                dma(pmb[:], t["pm_T"][bb])
                for tcn in range(NT):
                    pse = psB.tile([128, 512], F32, name="pse")
                    nc.tensor.matmul(pse[:], wcomb[:],
                                     imc[:, tcn * 512:(tcn + 1) * 512],
                                     start=True, stop=False)
                    nc.tensor.matmul(pse[:], ibf[:], pmb[:, tcn * 512:(tcn + 1) * 512],
                                     start=False, stop=True)
                    th = tanhpool.tile([128, 512], BF16, name="th", tag="tanh")
                    nc.scalar.activation(th[:], pse[:], Tanh, bias=pq_sb[:, bb:bb + 1])
                    nc.tensor.matmul(psE_t[:, tcn * 512:(tcn + 1) * 512],
                                     wvb[:, bb * b:(bb + 1) * b], th[:],
                                     start=False, stop=(bb == b - 1))

            # softmax (no max subtraction; masked lanes are exp(-1e30)=0)
            exw = spool.tile([b, T], F32, name="exw", tag="exw")
            sums = spool.tile([b, 1], F32, name="sums", tag="sums")
            nc.scalar.activation(exw[:], psE_t[:], Exp, accum_out=sums[:])
            recip = spool.tile([b, 1], F32, name="recip", tag="recip")
            nc.vector.reciprocal(recip[:], sums[:])
            nc.vector.tensor_scalar_mul(w_sb[:], exw[:], recip[:])
            dma(t["out2"][:], w_sb[:])

            # wT [t, b] chunks (bf16) for ctx matmuls
            for cc in range(NC):
                pt = psT2.tile([128, b], F32, name="pt", tag="pt")
                nc.tensor.transpose(pt[:], w_sb[:, cc * 128:(cc + 1) * 128],
                                    ief[0:b, 0:b])
                nc.scalar.activation(wT[:, cc * b:(cc + 1) * b], pt[:], Copy)

        # ---- ctx[b] = sum_t w[b,t] mem[b,t,:]  (per-b psum, rows != b garbage)
        ctx_bm = spool.tile([b, E], F32, name="ctx_bm", tag="ctx_bm")
        ctx_pack = consts.tile([128, KC * b], BF16, name="ctx_pack", tag="ctx_pack")
        with tc.tile_pool(name="psE2", bufs=4, space="PSUM") as psE2, \
             tc.tile_pool(name="psT3", bufs=2, space="PSUM") as psT3:
            for bb in range(b):
                mb = mempool.tile([128, NC * E], BF16, name="mb", tag="mem")
                dma(mb[:], bass.AP(t["mem"], bb * T * E,
                                   [[E, 128], [128 * E, NC], [1, E]]))
                pc = psE2.tile([b, E], F32, name="pc")
                for cc in range(NC):
                    nc.tensor.matmul(pc[:], wT[:, cc * b:(cc + 1) * b],
                                     mb[:, cc * E:(cc + 1) * E],
                                     start=(cc == 0), stop=(cc == NC - 1))
                nc.vector.copy_predicated(ctx_bm[:], mrows[:, bb * E:(bb + 1) * E],
                                          pc[:])
            transpose_pack(ctx_bm, KC, ctx_pack, psT3)

        # ================= phase C: decoder LSTM =================
        dh_pack = consts.tile([128, KH2 * b], BF16, name="dh_pack", tag="dh_pack")
        with tc.tile_pool(name="psA2", bufs=1, space="PSUM") as psA2, \
             tc.tile_pool(name="psT4", bufs=2, space="PSUM") as psT4:
            x2chunks = [ah_pack[:, kc * b:(kc + 1) * b] for kc in range(KH1)] + \
                       [ctx_pack[:, kc * b:(kc + 1) * b] for kc in range(KC)] + \
                       [hdp[:, kc * b:(kc + 1) * b] for kc in range(KH2)]
            dh_bm = lstm(x2chunks, "W_d", bd, G2, cds, KH1 + KC + KH2, psA2)
            transpose_pack(dh_bm, KH2, dh_pack, psT4)

        # ================= phase D: output heads =================
        with tc.tile_pool(name="psO", bufs=1, space="PSUM") as psO:
            x3chunks = [dh_pack[:, kc * b:(kc + 1) * b] for kc in range(KH2)] + \
                       [ctx_pack[:, kc * b:(kc + 1) * b] for kc in range(KC)]
            po = psO.tile([b, HO], F32, name="po")
            NK3 = KH2 + KC
            for kc in range(NK3):
                nc.tensor.matmul(po[:], x3chunks[kc], wo[:, kc * HO:(kc + 1) * HO],
                                 start=(kc == 0), stop=False)
            nc.tensor.matmul(po[:], ones[0:1, 0:b], bo[0:1, :], start=False, stop=True)
            o1 = spool.tile([b, HO], F32, name="o1", tag="o1")
            nc.scalar.activation(o1[:], po[:], Copy)
            dma(t["out1"][:], o1[:])


# ----------------------------------------------------------------------------
# public entry point
# ----------------------------------------------------------------------------

_NC_CACHE = {}


def _get_nc(cfg_key=None):
    if "nc" not in _NC_CACHE:
        _NC_CACHE["nc"] = build_kernel(FULL_CFG)
    return _NC_CACHE["nc"]


def kernel(**inputs):
    cfg = FULL_CFG
    nc = _get_nc()
    in_maps = make_in_maps(inputs, cfg)
    res = run_bass_kernel_spmd(nc, in_maps, core_ids=list(range(cfg["n_cores"])))
    return gather_outputs(res.results, cfg)


# revision 8
# speedup vs baseline: 1.0839x; 1.0839x over previous
"""Trainium2 Bass kernel for a Tacotron2-style decoder cell.

Strategy: data-parallel over batch B=64 across 8 NeuronCores (8 rows each),
all weights replicated in bf16.  Everything streamed through the PE in bf16,
accumulation + epilogues in f32.

Math reformulations (bit-compatible within bf16 tolerance):
  * location conv + location fc are fused:  Wcomb = W_lfc @ W_loc.reshape(F, 2K)
    so  loc_fc[b,t,:] = Wcomb @ im2col(aw)[b,:,t]  -- one matmul instead of two.
  * softmax computed as exp(e)/sum(exp(e)) without max subtraction (|e| <~ 3).
  * masked energies = energies + (mask ? -1e30 : 0), injected into the energies
    PSUM accumulation with an identity matmul.
  * biases folded into an extra k=1 matmul with a ones lhsT row.
"""

import numpy as np
import ml_dtypes

import concourse.bass as bass
import concourse.mybir as mybir
import concourse.tile as tile
from concourse import bacc
from concourse.bass_utils import run_bass_kernel_spmd

BF16 = mybir.dt.bfloat16
F32 = mybir.dt.float32
NEG_INF = -1e30

# ----------------------------------------------------------------------------
# configuration
# ----------------------------------------------------------------------------

FULL_CFG = dict(
    B=64, T=2048, E=256, A=128, P=128, R=1024, D=1024, H=256, F=32, K=31,
    n_cores=8,
)


def _derived(cfg):
    c = dict(cfg)
    c["b"] = c["B"] // c["n_cores"]       # local batch rows per core
    c["PAD"] = (c["K"] - 1) // 2
    c["CK"] = 2 * c["K"]                  # im2col rows
    c["G1"] = 4 * c["R"]                  # attention lstm gate width
    c["G2"] = 4 * c["D"]                  # decoder lstm gate width
    c["X1"] = c["P"] + c["E"]             # attention lstm input width
    c["X2"] = c["R"] + c["E"]             # decoder lstm input width (ah|ctx)
    c["HO"] = c["H"] + 1                  # proj columns + gate column
    for k in ("R", "D", "E", "X1"):
        assert c[k] % 128 == 0
    assert c["A"] == 128 and c["T"] % 512 == 0 and c["b"] <= 8
    return c


# ----------------------------------------------------------------------------
# host-side packing helpers
# ----------------------------------------------------------------------------

def _bf(x):
    return np.ascontiguousarray(np.asarray(x, np.float32)).astype(ml_dtypes.bfloat16)


def _f32(x):
    return np.ascontiguousarray(np.asarray(x, np.float32))


def _pack(x):
    """[b, n*128] row-major state -> [128, n*b] 'pack' layout.

    pack[p, c*b + j] = x[j, c*128 + p]  (chunk-major columns, batch minor)."""
    b, w = x.shape
    n = w // 128
    return np.ascontiguousarray(
        x.reshape(b, n, 128).transpose(2, 1, 0).reshape(128, n * b))


def make_in_maps(inputs, cfg):
    """Shard + pre-pack all inputs.  Returns list of per-core input dicts."""
    c = _derived(cfg)
    b, T, E, A, R, D, F, K = (c[k] for k in ("b", "T", "E", "A", "R", "D", "F", "K"))
    ncore = c["n_cores"]

    i = {k: np.asarray(v) for k, v in inputs.items()}

    # ---- replicated weights ----
    HB = 2048
    def wtiles(w):  # [KTOT*128, G] -> [KTOT, G//HB, 128, HB] contiguous tiles
        kt, g = w.shape[0] // 128, w.shape[1]
        nh = max(g // HB, 1)
        hb = g // nh
        return np.ascontiguousarray(
            w.reshape(kt, 128, nh, hb).transpose(0, 2, 1, 3))
    W_a = _bf(wtiles(np.concatenate([i["W_ih_a"].T, i["W_hh_a"].T], axis=0)))
    b_a = _bf((i["b_ih_a"] + i["b_hh_a"])[None, :])                        # [1, G1]
    W_d = _bf(wtiles(np.concatenate([i["W_ih_d"].T, i["W_hh_d"].T], axis=0)))
    b_d = _bf((i["b_ih_d"] + i["b_hh_d"])[None, :])                        # [1, G2]
    W_o = _bf(np.concatenate([i["W_proj"], i["W_gate"]], axis=0).T)        # [D+E, H+1]
    b_o = _bf(np.concatenate([i["b_proj"], i["b_gate"]])[None, :])         # [1, H+1]
    W_qT = _bf(np.asarray(i["W_q"]).T)                                     # [R, A]
    Wcomb = np.asarray(i["W_lfc"], np.float32) @ np.asarray(
        i["W_loc"], np.float32).reshape(F, 2 * K)                          # [A, 2K]
    WcombT = _bf(Wcomb.T)                                                  # [2K, A]
    Wv_bcols = np.zeros((A, 8 * b), np.float32)
    for j in range(b):
        Wv_bcols[:, j * b + j] = np.asarray(i["W_v"], np.float32)[0]
    Wv_bcols = _bf(Wv_bcols[:, : b * b])                                   # [A, b*b]
    eye_bf = _bf(np.eye(128, dtype=np.float32))                            # [128,128]
    eye_f = _f32(np.eye(128, dtype=np.float32))
    maskrows = np.zeros((b, b * E), np.uint8)
    for j in range(b):
        maskrows[j, j * E:(j + 1) * E] = 1                                 # [b, b*E]

    rep = dict(W_a=W_a, b_a=b_a, W_d=W_d, b_d=b_d, W_o=W_o, b_o=b_o,
               W_qT=W_qT, WcombT=WcombT, Wv_bcols=Wv_bcols,
               eye_bf=eye_bf, eye_f=eye_f, maskrows=maskrows)

    in_maps = []
    for core in range(ncore):
        rs = slice(core * b, (core + 1) * b)
        x1 = np.concatenate([i["last_frame"][rs], i["att_ctx"][rs]], axis=1)
        aw_pad = np.zeros((b, 2, T + 2 * c["PAD"]), np.float32)
        aw_pad[:, 0, c["PAD"]:c["PAD"] + T] = i["att_w"][rs]
        aw_pad[:, 1, c["PAD"]:c["PAD"] + T] = i["att_w_cum"][rs]
        maskadd = np.where(i["mask"][rs], NEG_INF, 0.0).astype(np.float32)
        m = dict(
            x1_pack=_bf(_pack(x1)),
            h1_pack=_bf(_pack(_f32(i["att_h"][rs]))),
            hd_pack=_bf(_pack(_f32(i["dec_h"][rs]))),
            c1_bm=_f32(i["att_c"][rs]),
            cd_bm=_f32(i["dec_c"][rs]),
            aw_pad=_bf(aw_pad),
            pm_T=_bf(np.asarray(i["processed_memory"][rs]).transpose(0, 2, 1)),
            mem=_bf(np.asarray(i["memory"][rs]).reshape(b, T // 128, 128, E)
                    .transpose(0, 2, 1, 3).reshape(b, 128, (T // 128) * E)),
            maskadd=_bf(maskadd),
        )
        m.update(rep)
        in_maps.append(m)
    return in_maps


def gather_outputs(results, cfg):
    c = _derived(cfg)
    H = c["H"]
    out1 = np.concatenate([r["out1"] for r in results], axis=0)   # [B, H+1]
    out2 = np.concatenate([r["out2"] for r in results], axis=0)   # [B, T]
    decoder_output = np.ascontiguousarray(out1[:, :H])
    stop = np.ascontiguousarray(out1[:, H:H + 1])
    weights = np.ascontiguousarray(out2)[:, None, :]
    return decoder_output, stop, weights


# ----------------------------------------------------------------------------
# kernel builder
# ----------------------------------------------------------------------------

def build_kernel(cfg):
    c = _derived(cfg)
    b, T, E, A, R, D = (c[k] for k in ("b", "T", "E", "A", "R", "D"))
    X1, X2, G1, G2, HO, CK, PAD = (c[k] for k in ("X1", "X2", "G1", "G2", "HO", "CK", "PAD"))
    TP = T + 2 * PAD
    NT = T // 512            # 512-wide t chunks
    NC = T // 128            # 128-wide t chunks
    NE = E // 128            # 128-wide e chunks
    K1 = X1 // 128           # x1 k-chunks
    KH1 = R // 128           # h k-chunks (lstm1)
    KH2 = D // 128
    KC = E // 128            # ctx k-chunks
    Sig = mybir.ActivationFunctionType.Sigmoid
    Tanh = mybir.ActivationFunctionType.Tanh
    Exp = mybir.ActivationFunctionType.Exp
    Copy = mybir.ActivationFunctionType.Copy

    nc = bacc.Bacc("TRN2", target_bir_lowering=False, debug=False)

    def din(name, shape, dt=BF16):
        return nc.declare_dram_parameter(name, list(shape), dt, isOutput=False)

    # inputs
    x1_pack = din("x1_pack", [128, K1 * b])
    h1_pack = din("h1_pack", [128, KH1 * b])
    hd_pack = din("hd_pack", [128, KH2 * b])
    c1_bm = din("c1_bm", [b, R], F32)
    cd_bm = din("cd_bm", [b, D], F32)
    aw_pad = din("aw_pad", [b, 2, TP])
    pm_T = din("pm_T", [b, A, T])
    mem = din("mem", [b, 128, (T // 128) * E])
    maskadd = din("maskadd", [b, T])
    HBa = min(G1, 2048); HBd = min(G2, 2048)
    W_a = din("W_a", [(X1 + R) // 128, G1 // HBa, 128, HBa])
    b_a = din("b_a", [1, G1])
    W_d = din("W_d", [(X2 + D) // 128, G2 // HBd, 128, HBd])
    b_d = din("b_d", [1, G2])
    W_o = din("W_o", [D + E, HO])
    b_o = din("b_o", [1, HO])
    W_qT = din("W_qT", [R, A])
    WcombT = din("WcombT", [CK, A])
    Wv_bcols = din("Wv_bcols", [A, b * b])
    eye_bf = din("eye_bf", [128, 128])
    eye_f = din("eye_f", [128, 128], F32)
    maskrows = din("maskrows", [b, b * E], mybir.dt.uint8)

    out1 = nc.declare_dram_parameter("out1", [b, HO], F32, isOutput=True)
    out2 = nc.declare_dram_parameter("out2", [b, T], F32, isOutput=True)

    with tile.TileContext(nc) as tc:
        _body(nc, tc, c, locals())
    nc.compile()
    return nc


def _body(nc, tc, c, t):
    b, T, E, A, R, D = (c[k] for k in ("b", "T", "E", "A", "R", "D"))
    X1, X2, G1, G2, HO, CK, PAD = (c[k] for k in ("X1", "X2", "G1", "G2", "HO", "CK", "PAD"))
    TP = T + 2 * PAD
    NT, NC, NE = T // 512, T // 128, E // 128
    K1, KH1, KH2, KC = X1 // 128, R // 128, D // 128, E // 128
    Sig = mybir.ActivationFunctionType.Sigmoid
    Tanh = mybir.ActivationFunctionType.Tanh
    Exp = mybir.ActivationFunctionType.Exp
    Copy = mybir.ActivationFunctionType.Copy

    HB = min(c["G1"], c["G2"], 2048)      # lstm psum half-block columns
    assert c["G1"] % HB == 0 and c["G2"] % HB == 0

    import contextlib
    ctx = contextlib.ExitStack()
    with ctx:
        # ---------------- persistent SBUF pools ----------------
        consts = ctx.enter_context(tc.tile_pool(name="consts", bufs=1))
        wpool = ctx.enter_context(tc.tile_pool(name="wpool", bufs=4))
        gpool = ctx.enter_context(tc.tile_pool(name="gpool", bufs=1))
        spool = ctx.enter_context(tc.tile_pool(name="spool", bufs=1))
        mempool = ctx.enter_context(tc.tile_pool(name="mempool", bufs=2))
        pmpool = ctx.enter_context(tc.tile_pool(name="pmpool", bufs=2))
        impool = ctx.enter_context(tc.tile_pool(name="impool", bufs=2))
        tanhpool = ctx.enter_context(tc.tile_pool(name="tanhpool", bufs=3))

        dma = nc.sync.dma_start

        # ---------------- constants / small inputs ----------------
        def lc(name, shape, dt=BF16, src=None):
            tl = consts.tile(list(shape), dt, name=name, tag=name)
            dma(tl[:], (t[name] if src is None else src)[:])
            return tl

        x1p = lc("x1_pack", [128, K1 * b])
        h1p = lc("h1_pack", [128, KH1 * b])
        hdp = lc("hd_pack", [128, KH2 * b])
        c1s = lc("c1_bm", [b, R], F32)
        cds = lc("cd_bm", [b, D], F32)
        madd = lc("maskadd", [b, T])
        wq = consts.tile([128, KH1 * A], BF16, name="wq", tag="wq")
        dma(wq[:], bass.AP(t["W_qT"], 0, [[A, 128], [128 * A, KH1], [1, A]]))
        wcomb = lc("WcombT", [CK, A])
        wvb = lc("Wv_bcols", [A, b * b])
        ibf = lc("eye_bf", [128, 128])
        ief = lc("eye_f", [128, 128], F32)
        mrows = lc("maskrows", [b, b * E], mybir.dt.uint8)
        wo = consts.tile([128, ((D + E) // 128) * HO], BF16, name="wo", tag="wo")
        dma(wo[:], bass.AP(t["W_o"], 0, [[HO, 128], [128 * HO, (D + E) // 128], [1, HO]]))
        ba = lc("b_a", [1, G1])
        bd = lc("b_d", [1, G2])
        bo = lc("b_o", [1, HO])
        ones = consts.tile([1, 8], BF16, name="ones", tag="ones")
        nc.vector.memset(ones[:], 1.0)

        # ---------------- generic lstm ----------------
        def lstm(xchunks, W_dram, b_sb, G, c_sb, KTOT, pspool):
            """xchunks: list of KTOT [128,b] lhsT APs. Returns h_bm [b, G/4] f32."""
            q = G // 4
            gact = gpool.tile([b, G], F32, name="gact", tag="gact")
            for hf in range(G // HB):
                ps = pspool.tile([b, HB], F32, name="ps_g", tag="ps_g")
                for kc in range(KTOT):
                    wt = wpool.tile([128, HB], BF16, name="wt", tag="w")
                    dma(wt[:], t[W_dram][kc, hf])
                    for nb in range(HB // 512):
                        nc.tensor.matmul(ps[:, nb * 512:(nb + 1) * 512], xchunks[kc],
                                         wt[:, nb * 512:(nb + 1) * 512],
                                         start=(kc == 0), stop=False)
                for nb in range(HB // 512):
                    g0 = hf * HB + nb * 512
                    nc.tensor.matmul(ps[:, nb * 512:(nb + 1) * 512], ones[0:1, 0:b],
                                     b_sb[0:1, g0:g0 + 512], start=False, stop=True)
                    func = Tanh if g0 // q == 2 else Sig
                    nc.scalar.activation(gact[:, g0:g0 + 512],
                                         ps[:, nb * 512:(nb + 1) * 512], func)
            sig_i, sig_f = gact[:, 0:q], gact[:, q:2 * q]
            tanh_g, sig_o = gact[:, 2 * q:3 * q], gact[:, 3 * q:4 * q]
            t1 = spool.tile([b, q], F32, name="t1", tag="t1")
            t2 = spool.tile([b, q], F32, name="t2", tag="t2")
            c2 = spool.tile([b, q], F32, name="c2", tag="c2")
            h_bm = spool.tile([b, q], F32, name=f"h_{W_dram}", tag=f"h_{W_dram}")
            nc.vector.tensor_mul(t1[:], sig_i, tanh_g)
            nc.vector.tensor_mul(t2[:], sig_f, c_sb[:])
            nc.vector.tensor_add(c2[:], t1[:], t2[:])
            nc.scalar.activation(t1[:], c2[:], Tanh)
            nc.vector.tensor_mul(h_bm[:], sig_o, t1[:])
            return h_bm

        def transpose_pack(src_bm, n128, dst_bf, pool):
            """src_bm [b, n128*128] f32 -> dst pack tile [128, n128*b] bf16."""
            for cc in range(n128):
                pt = pool.tile([128, b], F32, name="pt", tag="pt")
                nc.tensor.transpose(pt[:], src_bm[:, cc * 128:(cc + 1) * 128],
                                    ief[0:b, 0:b])
                nc.scalar.activation(dst_bf[:, cc * b:(cc + 1) * b], pt[:], Copy)

        # ================= phase A: attention LSTM =================
        ah_pack = consts.tile([128, KH1 * b], BF16, name="ah_pack", tag="ah_pack")
        with tc.tile_pool(name="psA1", bufs=1, space="PSUM") as psA1, \
             tc.tile_pool(name="psT1", bufs=2, space="PSUM") as psT1:
            x1chunks = [x1p[:, kc * b:(kc + 1) * b] for kc in range(K1)] + \
                       [h1p[:, kc * b:(kc + 1) * b] for kc in range(KH1)]
            ah_bm = lstm(x1chunks, "W_a", ba, G1, c1s, K1 + KH1, psA1)
            transpose_pack(ah_bm, KH1, ah_pack, psT1)

        # ================= phase B: attention =================
        wT = consts.tile([128, NC * b], BF16, name="wT", tag="wT")
        w_sb = spool.tile([b, T], F32, name="w_sb", tag="w_sb")
        with tc.tile_pool(name="psB", bufs=2, space="PSUM") as psB, \
             tc.tile_pool(name="psC", bufs=1, space="PSUM") as psC, \
             tc.tile_pool(name="psT2", bufs=2, space="PSUM") as psT2:
            # pq = W_q @ ah  -> [A, b]
            pq_ps = psT2.tile([128, b], F32, name="pt", tag="pt")
            for kc in range(KH1):
                nc.tensor.matmul(pq_ps[:], wq[:, kc * A:(kc + 1) * A],
                                 ah_pack[:, kc * b:(kc + 1) * b],
                                 start=(kc == 0), stop=(kc == KH1 - 1))
            pq_sb = spool.tile([128, b], F32, name="pq", tag="pq")
            nc.scalar.activation(pq_sb[:], pq_ps[:], Copy)

            # energies psum [b, T], seeded with the additive mask
            psE_t = psC.tile([b, T], F32, name="psE_t")
            for tcn in range(NT):
                nc.tensor.matmul(psE_t[:, tcn * 512:(tcn + 1) * 512], ibf[0:b, 0:b],
                                 madd[:, tcn * 512:(tcn + 1) * 512],
                                 start=True, stop=False)
            for bb in range(b):
                imc = impool.tile([CK, T], BF16, name="imc", tag="imc")
                dma(imc[:], bass.AP(t["aw_pad"], bb * 2 * TP,
                                    [[TP, 2], [1, c["K"]], [1, T]]))
                pmb = pmpool.tile([A, T], BF16, name="pmb", tag="pm")
                dma(pmb[:], t["pm_T"][bb])
                for tcn in range(NT):
                    pse = psB.tile([128, 512], F32, name="pse")
                    nc.tensor.matmul(pse[:], wcomb[:],
                                     imc[:, tcn * 512:(tcn + 1) * 512],
                                     start=True, stop=True)
                    nc.vector.tensor_add(pse[:], pse[:],
                                         pmb[:, tcn * 512:(tcn + 1) * 512])
                    th = tanhpool.tile([128, 512], BF16, name="th", tag="tanh")
                    nc.scalar.activation(th[:], pse[:], Tanh, bias=pq_sb[:, bb:bb + 1])
                    nc.tensor.matmul(psE_t[:, tcn * 512:(tcn + 1) * 512],
                                     wvb[:, bb * b:(bb + 1) * b], th[:],
                                     start=False, stop=(bb == b - 1))

            # softmax (no max subtraction; masked lanes are exp(-1e30)=0)
            exw = spool.tile([b, T], F32, name="exw", tag="exw")
            sums = spool.tile([b, 1], F32, name="sums", tag="sums")
            nc.scalar.activation(exw[:], psE_t[:], Exp, accum_out=sums[:])
            recip = spool.tile([b, 1], F32, name="recip", tag="recip")
            nc.vector.reciprocal(recip[:], sums[:])
            nc.vector.tensor_scalar_mul(w_sb[:], exw[:], recip[:])
            dma(t["out2"][:], w_sb[:])

            # wT [t, b] chunks (bf16) for ctx matmuls
            for cc in range(NC):
                pt = psT2.tile([128, b], F32, name="pt", tag="pt")
                nc.tensor.transpose(pt[:], w_sb[:, cc * 128:(cc + 1) * 128],
                                    ief[0:b, 0:b])
                nc.scalar.activation(wT[:, cc * b:(cc + 1) * b], pt[:], Copy)

        # ---- ctx[b] = sum_t w[b,t] mem[b,t,:]  (per-b psum, rows != b garbage)
        ctx_bm = spool.tile([b, E], F32, name="ctx_bm", tag="ctx_bm")
        ctx_pack = consts.tile([128, KC * b], BF16, name="ctx_pack", tag="ctx_pack")
        with tc.tile_pool(name="psE2", bufs=4, space="PSUM") as psE2, \
             tc.tile_pool(name="psT3", bufs=2, space="PSUM") as psT3:
            for bb in range(b):
                mb = mempool.tile([128, NC * E], BF16, name="mb", tag="mem")
                dma(mb[:], t["mem"][bb])
                pc = psE2.tile([b, E], F32, name="pc")
                for cc in range(NC):
                    nc.tensor.matmul(pc[:], wT[:, cc * b:(cc + 1) * b],
                                     mb[:, cc * E:(cc + 1) * E],
                                     start=(cc == 0), stop=(cc == NC - 1))
                nc.vector.copy_predicated(ctx_bm[:], mrows[:, bb * E:(bb + 1) * E],
                                          pc[:])
            transpose_pack(ctx_bm, KC, ctx_pack, psT3)

        # ================= phase C: decoder LSTM =================
        dh_pack = consts.tile([128, KH2 * b], BF16, name="dh_pack", tag="dh_pack")
        with tc.tile_pool(name="psA2", bufs=1, space="PSUM") as psA2, \
             tc.tile_pool(name="psT4", bufs=2, space="PSUM") as psT4:
            x2chunks = [ah_pack[:, kc * b:(kc + 1) * b] for kc in range(KH1)] + \
                       [ctx_pack[:, kc * b:(kc + 1) * b] for kc in range(KC)] + \
                       [hdp[:, kc * b:(kc + 1) * b] for kc in range(KH2)]
            dh_bm = lstm(x2chunks, "W_d", bd, G2, cds, KH1 + KC + KH2, psA2)
            transpose_pack(dh_bm, KH2, dh_pack, psT4)

        # ================= phase D: output heads =================
        with tc.tile_pool(name="psO", bufs=1, space="PSUM") as psO:
            x3chunks = [dh_pack[:, kc * b:(kc + 1) * b] for kc in range(KH2)] + \
                       [ctx_pack[:, kc * b:(kc + 1) * b] for kc in range(KC)]
            po = psO.tile([b, HO], F32, name="po")
            NK3 = KH2 + KC
            for kc in range(NK3):
                nc.tensor.matmul(po[:], x3chunks[kc], wo[:, kc * HO:(kc + 1) * HO],
                                 start=(kc == 0), stop=False)
            nc.tensor.matmul(po[:], ones[0:1, 0:b], bo[0:1, :], start=False, stop=True)
            o1 = spool.tile([b, HO], F32, name="o1", tag="o1")
            nc.scalar.activation(o1[:], po[:], Copy)
            dma(t["out1"][:], o1[:])


# ----------------------------------------------------------------------------
# public entry point
# ----------------------------------------------------------------------------

_NC_CACHE = {}


def _get_nc(cfg_key=None):
    if "nc" not in _NC_CACHE:
        _NC_CACHE["nc"] = build_kernel(FULL_CFG)
    return _NC_CACHE["nc"]


def kernel(**inputs):
    cfg = FULL_CFG
    nc = _get_nc()
    in_maps = make_in_maps(inputs, cfg)
    res = run_bass_kernel_spmd(nc, in_maps, core_ids=list(range(cfg["n_cores"])))
    return gather_outputs(res.results, cfg)


# revision 16
# speedup vs baseline: 1.1236x; 1.0367x over previous
"""Trainium2 Bass kernel for a Tacotron2-style decoder cell (B=64, T=2048).

Distribution across the 8 NeuronCores of one TRN2 chip:
  * LSTM cells run MODEL-PARALLEL: each core owns a 128-wide slice of the
    hidden dim of both LSTMCells (weights sharded 8x, full batch locally),
    then a tiny AllGather (32 KB) rebuilds the full hidden state everywhere.
  * the location-sensitive attention runs DATA-PARALLEL over batch (8 rows
    per core): memory / processed_memory / attention-weight state are
    batch-sharded; the per-core context vectors are AllGathered for the
    decoder LSTM.
  * the output heads are computed redundantly on every core (full batch,
    ~1 us) to avoid any rank-dependent slicing.

Everything streamed through the PE is bf16; accumulation + epilogues f32.

Math reformulations (validated to bf16 tolerance against the reference):
  * conv + location-fc fused:  Wcomb = W_lfc @ W_loc.reshape(F, 2K), applied
    to an im2col gather of the padded attention weights.
  * softmax as exp(e)/sum(exp(e)) without max subtraction (|energies| < ~3;
    masked lanes get -1e30 injected into the PSUM accumulation, exp -> 0).
  * biases folded in as an extra k=1 matmul against a ones row.
"""

import numpy as np
import ml_dtypes

import concourse.bass as bass
import concourse.mybir as mybir
import concourse.tile as tile
from concourse import bacc
from concourse.bass_utils import run_bass_kernel_spmd

BF16 = mybir.dt.bfloat16
F32 = mybir.dt.float32
I32 = mybir.dt.int32
NEG_INF = -1e30

DEBUG_NO_CC = False      # replace collectives with local DMA (hang bisect)
DEBUG_STATIC_ROFF = False  # replace dynamic rank slice with [0:b] (hang bisect)

FULL_CFG = dict(
    B=64, T=2048, E=256, A=128, P=128, R=1024, D=1024, H=256, F=32, K=31,
    n_cores=8,
)


def _derived(cfg):
    c = dict(cfg)
    n = c["n_cores"]
    c["b"] = c["B"] // n              # local batch rows (attention shard)
    c["PAD"] = (c["K"] - 1) // 2
    c["CK"] = 2 * c["K"]
    c["RS"] = c["R"] // n             # hidden shard (lstm1)
    c["DS"] = c["D"] // n             # hidden shard (lstm2)
    c["G1S"] = 4 * c["RS"]
    c["G2S"] = 4 * c["DS"]
    c["X1"] = c["P"] + c["E"]
    c["X2"] = c["R"] + c["E"]
    c["HO"] = c["H"] + 1
    for k in ("R", "D", "E", "X1", "RS", "DS"):
        assert c[k] % 128 == 0
    assert c["A"] == 128 and c["T"] % 512 == 0 and c["b"] <= 8
    assert c["B"] <= 128
    return c


# ----------------------------------------------------------------------------
# host-side packing
# ----------------------------------------------------------------------------

def _bf(x):
    return np.ascontiguousarray(np.asarray(x, np.float32)).astype(ml_dtypes.bfloat16)


def _f32(x):
    return np.ascontiguousarray(np.asarray(x, np.float32))


def _pack(x):
    """[B, n*128] row-major -> [128, n*B]: pack[p, c*B + j] = x[j, c*128 + p]."""
    bb, w = x.shape
    nch = w // 128
    return np.ascontiguousarray(
        x.reshape(bb, nch, 128).transpose(2, 1, 0).reshape(128, nch * bb))


def _wtiles(w):
    """[KT*128, G] -> [KT, 128, G] contiguous k-chunk tiles."""
    kt = w.shape[0] // 128
    return np.ascontiguousarray(w.reshape(kt, 128, w.shape[1]))


def make_in_maps(inputs, cfg):
    c = _derived(cfg)
    n = c["n_cores"]
    B, b, T, E, A, R, D, F, K = (c[k] for k in ("B", "b", "T", "E", "A", "R", "D", "F", "K"))
    RS, DS = c["RS"], c["DS"]

    i = {k: np.asarray(v) for k, v in inputs.items()}

    W_aT = _f32(np.concatenate([i["W_ih_a"].T, i["W_hh_a"].T], axis=0))  # [X1+R, 4R]
    b_a = _f32(i["b_ih_a"] + i["b_hh_a"])[None, :]
    W_dT = _f32(np.concatenate([i["W_ih_d"].T, i["W_hh_d"].T], axis=0))  # [X2+D, 4D]
    b_d = _f32(i["b_ih_d"] + i["b_hh_d"])[None, :]

    def shard_cols(w, sh, k):
        # columns of the 4 gate blocks belonging to hidden slice k
        return np.ascontiguousarray(
            w.reshape(w.shape[0], 4, n, sh)[:, :, k, :].reshape(w.shape[0], 4 * sh))

    W_qT = _bf(np.asarray(i["W_q"]).T)
    Wcomb = np.asarray(i["W_lfc"], np.float32) @ np.asarray(
        i["W_loc"], np.float32).reshape(F, 2 * K)
    WcombT = _bf(Wcomb.T)                                                # [2K, A]
    Wv_bcols = np.zeros((A, b * b), np.float32)
    for j in range(b):
        Wv_bcols[:, j * b + j] = np.asarray(i["W_v"], np.float32)[0]
    Wv_bcols = _bf(Wv_bcols)
    eye_bf = _bf(np.eye(128, dtype=np.float32))
    eye_f = _f32(np.eye(128, dtype=np.float32))
    maskrows = np.zeros((b, b * E), np.uint8)
    for j in range(b):
        maskrows[j, j * E:(j + 1) * E] = 1

    x1 = np.concatenate([i["last_frame"], i["att_ctx"]], axis=1)         # [B, X1]
    rep = dict(
        x1_pack=_bf(_pack(_f32(x1))),
        h1_pack=_bf(_pack(_f32(i["att_h"]))),
        hd_pack=_bf(_pack(_f32(i["dec_h"]))),
        W_o=_bf(_wtiles(np.asarray(
            np.concatenate([i["W_proj"], i["W_gate"]], axis=0).T, np.float32))),
        b_o=_bf(np.concatenate([i["b_proj"], i["b_gate"]])[None, :]),
        W_qT=W_qT, WcombT=WcombT, Wv_bcols=Wv_bcols,
        eye_bf=eye_bf, eye_f=eye_f, maskrows=maskrows,
    )

    in_maps = []
    for k in range(n):
        rs = slice(k * b, (k + 1) * b)
        aw_pad = np.zeros((b, 2, T + 2 * c["PAD"]), np.float32)
        aw_pad[:, 0, c["PAD"]:c["PAD"] + T] = i["att_w"][rs]
        aw_pad[:, 1, c["PAD"]:c["PAD"] + T] = i["att_w_cum"][rs]
        maskadd = np.where(i["mask"][rs], NEG_INF, 0.0).astype(np.float32)
        m = dict(
            W_a=_bf(_wtiles(shard_cols(W_aT, RS, k))),         # [KT1,128,G1S]
            b_a=_bf(shard_cols(b_a, RS, k)),                   # [1, G1S]
            W_d=_bf(_wtiles(shard_cols(W_dT, DS, k))),         # [KT2,128,G2S]
            b_d=_bf(shard_cols(b_d, DS, k)),                   # [1, G2S]
            c1_mp=_f32(i["att_c"][:, k * RS:(k + 1) * RS]),    # [B, RS]
            cd_mp=_f32(i["dec_c"][:, k * DS:(k + 1) * DS]),    # [B, DS]
            rankoff=np.array([[k * b]], np.int32),
            selmat=_bf(np.eye(B, dtype=np.float32)[:, k * b:(k + 1) * b]),
            aw_pad=_bf(aw_pad),
            pm_T=_bf(np.asarray(i["processed_memory"][rs]).transpose(0, 2, 1)),
            mem=_bf(np.asarray(i["memory"][rs]).reshape(b, T // 128, 128, E)
                    .transpose(0, 2, 1, 3).reshape(b, 128, (T // 128) * E)),
            maskadd=_bf(maskadd),
        )
        m.update(rep)
        in_maps.append(m)
    return in_maps


def gather_outputs(results, cfg):
    c = _derived(cfg)
    H = c["H"]
    out1 = results[0]["out1"]                                     # [B, HO] (full)
    out2 = np.concatenate([r["out2"] for r in results], axis=0)   # [B, T]
    decoder_output = np.ascontiguousarray(out1[:, :H])
    stop = np.ascontiguousarray(out1[:, H:H + 1])
    weights = np.ascontiguousarray(out2)[:, None, :]
    return decoder_output, stop, weights


# ----------------------------------------------------------------------------
# kernel builder
# ----------------------------------------------------------------------------

def build_kernel(cfg):
    c = _derived(cfg)
    B, b, T, E, A, R, D = (c[k] for k in ("B", "b", "T", "E", "A", "R", "D"))
    X1, X2, HO, CK, PAD = (c[k] for k in ("X1", "X2", "HO", "CK", "PAD"))
    RS, DS, G1S, G2S = (c[k] for k in ("RS", "DS", "G1S", "G2S"))
    TP = T + 2 * PAD
    KT1 = (X1 + R) // 128
    KT2 = (X2 + D) // 128

    nc = bacc.Bacc("TRN2", target_bir_lowering=False, debug=False)

    def din(name, shape, dt=BF16):
        return nc.declare_dram_parameter(name, list(shape), dt, isOutput=False)

    x1_pack = din("x1_pack", [128, (X1 // 128) * B])
    h1_pack = din("h1_pack", [128, (R // 128) * B])
    hd_pack = din("hd_pack", [128, (D // 128) * B])
    c1_mp = din("c1_mp", [B, RS], F32)
    cd_mp = din("cd_mp", [B, DS], F32)
    rankoff = din("rankoff", [1, 1], I32)
    selmat = din("selmat", [B, b])
    aw_pad = din("aw_pad", [b, 2, TP])
    pm_T = din("pm_T", [b, A, T])
    mem = din("mem", [b, 128, (T // 128) * E])
    maskadd = din("maskadd", [b, T])
    W_a = din("W_a", [KT1, 128, G1S])
    b_a = din("b_a", [1, G1S])
    W_d = din("W_d", [KT2, 128, G2S])
    b_d = din("b_d", [1, G2S])
    W_o = din("W_o", [(D + E) // 128, 128, HO])
    b_o = din("b_o", [1, HO])
    W_qT = din("W_qT", [R, A])
    WcombT = din("WcombT", [CK, A])
    Wv_bcols = din("Wv_bcols", [A, b * b])
    eye_bf = din("eye_bf", [128, 128])
    eye_f = din("eye_f", [128, 128], F32)
    maskrows = din("maskrows", [b, b * E], mybir.dt.uint8)

    out1 = nc.declare_dram_parameter("out1", [B, HO], F32, isOutput=True)
    out2 = nc.declare_dram_parameter("out2", [b, T], F32, isOutput=True)

    n = c["n_cores"]
    NS1, NS2 = RS // 128, DS // 128
    ag_in_ah = nc.dram_tensor("ag_in_ah", [NS1 * 128, B], BF16)
    ag_out_ah = nc.dram_tensor("ag_out_ah", [n * NS1 * 128, B], BF16)
    ag_in_ctx = nc.dram_tensor("ag_in_ctx", [b, E], BF16)
    ag_out_ctx = nc.dram_tensor("ag_out_ctx", [B, E], BF16)
    ag_in_dh = nc.dram_tensor("ag_in_dh", [NS2 * 128, B], BF16)
    ag_out_dh = nc.dram_tensor("ag_out_dh", [n * NS2 * 128, B], BF16)

    with tile.TileContext(nc) as tc:
        _body(nc, tc, c, locals())
    nc.compile()
    return nc


def _body(nc, tc, c, t):
    n = c["n_cores"]
    B, b, T, E, A, R, D = (c[k] for k in ("B", "b", "T", "E", "A", "R", "D"))
    X1, X2, HO, CK, PAD = (c[k] for k in ("X1", "X2", "HO", "CK", "PAD"))
    RS, DS, G1S, G2S = (c[k] for k in ("RS", "DS", "G1S", "G2S"))
    TP = T + 2 * PAD
    NT, NC = T // 512, T // 128
    KT1, KT2 = (X1 + R) // 128, (X2 + D) // 128
    KH1, KH2, KC = R // 128, D // 128, E // 128
    NS1, NS2 = RS // 128, DS // 128          # shard chunks per core
    Sig = mybir.ActivationFunctionType.Sigmoid
    Tanh = mybir.ActivationFunctionType.Tanh
    Exp = mybir.ActivationFunctionType.Exp
    Copy = mybir.ActivationFunctionType.Copy
    RG = [list(range(n))]

    import contextlib
    ctx = contextlib.ExitStack()
    with ctx:
        consts = ctx.enter_context(tc.tile_pool(name="consts", bufs=1))
        wpool = ctx.enter_context(tc.tile_pool(name="wpool", bufs=4))
        gpool = ctx.enter_context(tc.tile_pool(name="gpool", bufs=1))
        spool = ctx.enter_context(tc.tile_pool(name="spool", bufs=1))
        mempool = ctx.enter_context(tc.tile_pool(name="mempool", bufs=2))
        pmpool = ctx.enter_context(tc.tile_pool(name="pmpool", bufs=2))
        impool = ctx.enter_context(tc.tile_pool(name="impool", bufs=2))
        tanhpool = ctx.enter_context(tc.tile_pool(name="tanhpool", bufs=3))
        dma = nc.sync.dma_start
        agsem_d = nc.alloc_semaphore("agsem_d")
        agsem_c = nc.alloc_semaphore("agsem_c")
        agcnt = {"d": 0, "c": 0}

        def run_allgather(src_tile_ap, gin_dst_ap, gin, gout, dst_tile_ap,
                          dst_src_ap):
            with tc.tile_critical():
                nc.gpsimd.dma_start(out=gin_dst_ap, in_=src_tile_ap).then_inc(
                    agsem_d, 16)
                agcnt["d"] += 16
                nc.gpsimd.wait_ge(agsem_d, agcnt["d"])
                if DEBUG_NO_CC:
                    sz = int(np.prod(gin.shape))
                    nc.gpsimd.dma_start(
                        out=bass.AP(gout, 0, [[1, sz]]),
                        in_=bass.AP(gin, 0, [[1, sz]])).then_inc(agsem_c, 16)
                    agcnt["c"] += 16
                else:
                    nc.gpsimd.collective_compute(
                        "AllGather", mybir.AluOpType.bypass,
                        ins=[gin.ap().opt()], outs=[gout.ap().opt()],
                        replica_groups=RG).then_inc(agsem_c, 1)
                    agcnt["c"] += 1
                nc.gpsimd.wait_ge(agsem_c, agcnt["c"])
                nc.gpsimd.dma_start(out=dst_tile_ap, in_=dst_src_ap).then_inc(
                    agsem_d, 16)
                agcnt["d"] += 16
                nc.gpsimd.wait_ge(agsem_d, agcnt["d"])

        def lc(name, shape, dt=BF16):
            tl = consts.tile(list(shape), dt, name=name, tag=name)
            dma(tl[:], t[name][:])
            return tl

        x1p = lc("x1_pack", [128, (X1 // 128) * B])
        h1p = lc("h1_pack", [128, KH1 * B])
        hdp = lc("hd_pack", [128, KH2 * B])
        c1s = lc("c1_mp", [B, RS], F32)
        cds = lc("cd_mp", [B, DS], F32)
        rko = lc("rankoff", [1, 1], I32)
        sel = lc("selmat", [B, b])
        madd = lc("maskadd", [b, T])
        wq = consts.tile([128, KH1 * A], BF16, name="wq", tag="wq")
        dma(wq[:], bass.AP(t["W_qT"], 0, [[A, 128], [128 * A, KH1], [1, A]]))
        wcomb = lc("WcombT", [CK, A])
        wvb = lc("Wv_bcols", [A, b * b])
        ibf = lc("eye_bf", [128, 128])
        ief = lc("eye_f", [128, 128], F32)
        mrows = lc("maskrows", [b, b * E], mybir.dt.uint8)
        wo = consts.tile([128, ((D + E) // 128) * HO], BF16, name="wo", tag="wo")
        dma(wo[:], bass.AP(t["W_o"], 0, [[HO, 128], [128 * HO, (D + E) // 128],
                                         [1, HO]]))
        ba = lc("b_a", [1, G1S])
        bd = lc("b_d", [1, G2S])
        bo = lc("b_o", [1, HO])
        ones = consts.tile([1, 128], BF16, name="ones", tag="ones")
        nc.vector.memset(ones[:], 1.0)

        # ---------------- helpers ----------------
        def mp_lstm(xchunks, W_dram, b_sb, GS, c_sb, KTOT, SH, pspool):
            """Model-parallel LSTM shard: full batch B, gate width GS = 4*SH.
            Returns h_sh [B, SH] f32."""
            gact = gpool.tile([B, GS], F32, name="gact", tag="gact")
            ps = pspool.tile([B, GS], F32, name="ps_g", tag="ps_g")
            NB = max(GS // 512, 1)
            W512 = GS // NB
            for kc in range(KTOT):
                wt = wpool.tile([128, GS], BF16, name="wt", tag="w")
                dma(wt[:], t[W_dram][kc])
                for nb in range(NB):
                    nc.tensor.matmul(ps[:, nb * W512:(nb + 1) * W512], xchunks[kc],
                                     wt[:, nb * W512:(nb + 1) * W512],
                                     start=(kc == 0), stop=False)
            for nb in range(NB):
                nc.tensor.matmul(ps[:, nb * W512:(nb + 1) * W512], ones[0:1, 0:B],
                                 b_sb[0:1, nb * W512:(nb + 1) * W512],
                                 start=False, stop=True)
            # evictions: i | f sigmoid, g tanh, o sigmoid (blocks of width SH)
            for g0, g1, func in ((0, 2 * SH, Sig), (2 * SH, 3 * SH, Tanh),
                                 (3 * SH, 4 * SH, Sig)):
                nc.scalar.activation(gact[:, g0:g1], ps[:, g0:g1], func)
            sig_i, sig_f = gact[:, 0:SH], gact[:, SH:2 * SH]
            tanh_g, sig_o = gact[:, 2 * SH:3 * SH], gact[:, 3 * SH:4 * SH]
            t1 = spool.tile([B, SH], F32, name="t1", tag="t1")
            t2 = spool.tile([B, SH], F32, name="t2", tag="t2")
            c2 = spool.tile([B, SH], F32, name="c2", tag="c2")
            h_sh = spool.tile([B, SH], F32, name=f"h_{W_dram}", tag=f"h_{W_dram}")
            nc.vector.tensor_mul(t1[:], sig_i, tanh_g)
            nc.vector.tensor_mul(t2[:], sig_f, c_sb[:])
            nc.vector.tensor_add(c2[:], t1[:], t2[:])
            nc.scalar.activation(t1[:], c2[:], Tanh)
            nc.vector.tensor_mul(h_sh[:], sig_o, t1[:])
            return h_sh

        def shard_ag(h_sh, NS, dst_pack, pool, gtag):
            """transpose h_sh [B, NS*128] -> [NS*128, B] bf16, AllGather ->
            dst_pack [128, n*NS*B] (global chunk-major pack)."""
            hT = spool.tile([128, NS * B], BF16, name=f"hT_{gtag}", tag=f"hT_{gtag}")
            for cc in range(NS):
                pt = pool.tile([128, B], F32, name="pt", tag="pt")
                nc.tensor.transpose(pt[:], h_sh[:, cc * 128:(cc + 1) * 128],
                                    ief[0:B, 0:B])
                nc.scalar.activation(hT[:, cc * B:(cc + 1) * B], pt[:], Copy)
            gin, gout = t[f"ag_in_{gtag}"], t[f"ag_out_{gtag}"]
            run_allgather(
                hT[:], bass.AP(gin, 0, [[B, 128], [128 * B, NS], [1, B]]),
                gin, gout, dst_pack[:],
                bass.AP(gout, 0, [[B, 128], [128 * B, n * NS], [1, B]]))

        # ================= phase A: attention LSTM (model-parallel) ========
        ah_pack = consts.tile([128, KH1 * B], BF16, name="ah_pack", tag="ah_pack")
        with tc.tile_pool(name="psA1", bufs=1, space="PSUM") as psA1, \
             tc.tile_pool(name="psT1", bufs=2, space="PSUM") as psT1:
            x1chunks = [x1p[:, kc * B:(kc + 1) * B] for kc in range(X1 // 128)] + \
                       [h1p[:, kc * B:(kc + 1) * B] for kc in range(KH1)]
            ah_sh = mp_lstm(x1chunks, "W_a", ba, G1S, c1s, KT1, RS, psA1)
            shard_ag(ah_sh, NS1, ah_pack, psT1, "ah")

        # ================= phase B: attention (batch-parallel) =============
        wT = consts.tile([128, NC * b], BF16, name="wT", tag="wT")
        w_sb = spool.tile([b, T], F32, name="w_sb", tag="w_sb")
        # pq = W_q @ ah (full batch), then select our b columns via a
        # per-core one-hot matmul (static program, rank-dependent data only)
        pq_sb = spool.tile([128, b], F32, name="pq", tag="pq")
        with tc.tile_pool(name="psQ", bufs=1, space="PSUM") as psQ:
            pq_ps = psQ.tile([128, B], F32, name="pq_ps", tag="pq_ps")
            for kc in range(KH1):
                nc.tensor.matmul(pq_ps[:], wq[:, kc * A:(kc + 1) * A],
                                 ah_pack[:, kc * B:(kc + 1) * B],
                                 start=(kc == 0), stop=(kc == KH1 - 1))
            pq_all = spool.tile([128, B], BF16, name="pq_all", tag="pq_all")
            nc.scalar.activation(pq_all[:], pq_ps[:], Copy)
            ptq = psQ.tile([B, 128], BF16, name="ptq", tag="ptq")
            nc.tensor.transpose(ptq[:], pq_all[:], ibf[:, :])
            pq_rows = spool.tile([B, 128], BF16, name="pq_rows", tag="pq_rows")
            nc.scalar.activation(pq_rows[:], ptq[:], Copy)
            pq_ps2 = psQ.tile([128, b], F32, name="pq_ps2", tag="pq_ps2")
            nc.tensor.matmul(pq_ps2[:], pq_rows[:], sel[:], start=True, stop=True)
            nc.scalar.activation(pq_sb[:], pq_ps2[:], Copy)

        with tc.tile_pool(name="psB", bufs=2, space="PSUM") as psB, \
             tc.tile_pool(name="psC", bufs=1, space="PSUM") as psC, \
             tc.tile_pool(name="psT2", bufs=2, space="PSUM") as psT2:
            psE_t = psC.tile([b, T], F32, name="psE_t")
            for tcn in range(NT):
                nc.tensor.matmul(psE_t[:, tcn * 512:(tcn + 1) * 512], ibf[0:b, 0:b],
                                 madd[:, tcn * 512:(tcn + 1) * 512],
                                 start=True, stop=False)
            for bb in range(b):
                imc = impool.tile([CK, T], BF16, name="imc", tag="imc")
                dma(imc[:], bass.AP(t["aw_pad"], bb * 2 * TP,
                                    [[TP, 2], [1, c["K"]], [1, T]]))
                pmb = pmpool.tile([A, T], BF16, name="pmb", tag="pm")
                dma(pmb[:], t["pm_T"][bb])
                for tcn in range(NT):
                    pse = psB.tile([128, 512], F32, name="pse")
                    nc.tensor.matmul(pse[:], wcomb[:],
                                     imc[:, tcn * 512:(tcn + 1) * 512],
                                     start=True, stop=True)
                    nc.vector.tensor_add(pse[:], pse[:],
                                         pmb[:, tcn * 512:(tcn + 1) * 512])
                    th = tanhpool.tile([128, 512], BF16, name="th", tag="tanh")
                    nc.scalar.activation(th[:], pse[:], Tanh, bias=pq_sb[:, bb:bb + 1])
                    nc.tensor.matmul(psE_t[:, tcn * 512:(tcn + 1) * 512],
                                     wvb[:, bb * b:(bb + 1) * b], th[:],
                                     start=False, stop=(bb == b - 1))

            exw = spool.tile([b, T], F32, name="exw", tag="exw")
            sums = spool.tile([b, 1], F32, name="sums", tag="sums")
            nc.scalar.activation(exw[:], psE_t[:], Exp, accum_out=sums[:])
            recip = spool.tile([b, 1], F32, name="recip", tag="recip")
            nc.vector.reciprocal(recip[:], sums[:])
            nc.vector.tensor_scalar_mul(w_sb[:], exw[:], recip[:])
            dma(t["out2"][:], w_sb[:])

            for cc in range(NC):
                pt = psT2.tile([128, b], F32, name="pt", tag="pt")
                nc.tensor.transpose(pt[:], w_sb[:, cc * 128:(cc + 1) * 128],
                                    ief[0:b, 0:b])
                nc.scalar.activation(wT[:, cc * b:(cc + 1) * b], pt[:], Copy)

        # ---- ctx + AllGather to full batch ----
        ctx_pack = consts.tile([128, KC * B], BF16, name="ctx_pack", tag="ctx_pack")
        with tc.tile_pool(name="psE2", bufs=4, space="PSUM") as psE2, \
             tc.tile_pool(name="psT3", bufs=2, space="PSUM") as psT3:
            ctx_bm = spool.tile([b, E], BF16, name="ctx_bm", tag="ctx_bm")
            for bb in range(b):
                mb = mempool.tile([128, NC * E], BF16, name="mb", tag="mem")
                dma(mb[:], t["mem"][bb])
                pc = psE2.tile([b, E], F32, name="pc")
                for cc in range(NC):
                    nc.tensor.matmul(pc[:], wT[:, cc * b:(cc + 1) * b],
                                     mb[:, cc * E:(cc + 1) * E],
                                     start=(cc == 0), stop=(cc == NC - 1))
                nc.vector.copy_predicated(ctx_bm[:], mrows[:, bb * E:(bb + 1) * E],
                                          pc[:])
            # AllGather ctx rows -> [B, E], then transpose to pack layout
            ctx64 = spool.tile([B, E], BF16, name="ctx64", tag="ctx64")
            run_allgather(ctx_bm[:], t["ag_in_ctx"].ap(), t["ag_in_ctx"],
                          t["ag_out_ctx"], ctx64[:], t["ag_out_ctx"].ap())
            for cc in range(KC):
                pt = psT3.tile([128, B], BF16, name="ptb", tag="ptb")
                nc.tensor.transpose(pt[:], ctx64[:, cc * 128:(cc + 1) * 128],
                                    ibf[0:B, 0:B])
                nc.scalar.activation(ctx_pack[:, cc * B:(cc + 1) * B], pt[:], Copy)

        # ================= phase C: decoder LSTM (model-parallel) ==========
        dh_pack = consts.tile([128, KH2 * B], BF16, name="dh_pack", tag="dh_pack")
        with tc.tile_pool(name="psA2", bufs=1, space="PSUM") as psA2, \
             tc.tile_pool(name="psT4", bufs=2, space="PSUM") as psT4:
            x2chunks = [ah_pack[:, kc * B:(kc + 1) * B] for kc in range(KH1)] + \
                       [ctx_pack[:, kc * B:(kc + 1) * B] for kc in range(KC)] + \
                       [hdp[:, kc * B:(kc + 1) * B] for kc in range(KH2)]
            dh_sh = mp_lstm(x2chunks, "W_d", bd, G2S, cds, KT2, DS, psA2)
            shard_ag(dh_sh, NS2, dh_pack, psT4, "dh")

        # ================= phase D: output heads (full batch, redundant) ===
        with tc.tile_pool(name="psO", bufs=1, space="PSUM") as psO:
            x3chunks = [dh_pack[:, kc * B:(kc + 1) * B] for kc in range(KH2)] + \
                       [ctx_pack[:, kc * B:(kc + 1) * B] for kc in range(KC)]
            po = psO.tile([B, HO], F32, name="po")
            for kc in range(KH2 + KC):
                nc.tensor.matmul(po[:], x3chunks[kc], wo[:, kc * HO:(kc + 1) * HO],
                                 start=(kc == 0), stop=False)
            nc.tensor.matmul(po[:], ones[0:1, 0:B], bo[0:1, :], start=False,
                             stop=True)
            o1 = spool.tile([B, HO], F32, name="o1", tag="o1")
            nc.scalar.activation(o1[:], po[:], Copy)
            dma(t["out1"][:], o1[:])


# ----------------------------------------------------------------------------
# public entry point
# ----------------------------------------------------------------------------

_NC_CACHE = {}


def _get_nc():
    if "nc" not in _NC_CACHE:
        _NC_CACHE["nc"] = build_kernel(FULL_CFG)
    return _NC_CACHE["nc"]


def kernel(**inputs):
    cfg = FULL_CFG
    nc = _get_nc()
    in_maps = make_in_maps(inputs, cfg)
    res = run_bass_kernel_spmd(nc, in_maps, core_ids=list(range(cfg["n_cores"])))
    return gather_outputs(res.results, cfg)
